# revision 22
# baseline (speedup 1.0000x reference)
"""MoE grouped-GEMM expert MLP for Trainium2, expert-parallel over 8 NeuronCores.

Problem: x:(B=2, E=8, N=2048, D=1024), per-expert 2-layer GELU MLP with
w1:(E, D, F=4096), w2:(E, F, D).  Reference computes
  xe = x.reshape(E, B*N, D)          # pure buffer reinterpretation
  h  = gelu_tanh(xe @ w1 + b1)
  out= h @ w2 + b2                   # reshaped back to (B, E, N, D)

Sharding: expert parallelism — core e runs expert e on its contiguous
token block xe[e] (4096 tokens).  No collectives needed.

Per-core layout: hidden activations kept transposed ("hT" = [f, tok]) so both
weight matrices are consumed in their NATIVE layouts:
  GEMM1: hT[f,tok]  = (w1[d,f] as lhsT).T @ xT[d,tok]
  GEMM2: out[tok,d] = (hT[f,tok] slice as lhsT).T @ w2[f,d]

Profiling facts this schedule is built around (from NTFF DMA packet data):
  - DMA packets that cast fp32->bf16 in flight run at roughly HALF the DMA
    engine byte rate (~150 GB/s aggregate vs ~280 GB/s raw).
  - XBAR DMA-transposes are much slower still (~10-13 GB/s per transfer):
    one token-chunk's transposes are ~100us of queue time, which is what
    actually paced the 984us baseline (8 transposes/chunk on one queue).
Consequences:
  - No DMA transposes at all.  Every chunk's xT is produced by PE-mode
    transposes (bf16, 1 cycle/row: ~1.7us/chunk of PE time).  x streams in
    as fp32->bf16 cast-DMAs on SWDGE into a small SBUF staging pool
    (chunk 0: raw fp32 on the sync queue + fp32 PE transpose, to start the
    PE ~10us in instead of waiting ~15us for a cast).
  - w1/w2 are DMA'd as RAW fp32 slices into small staging tiles — w1 g0-g5
    + w2 g3-g7 as eighth-groups on the scalar HWDGE queue, w1 g6-g7 +
    w2 g0-g2 as quarter-groups on the sync queue — and cast to resident
    bf16 tiles by the otherwise-idle Vector engine.
  - GEMM2 of chunk 0 is emitted f-group-major (all 8 PSUM tiles accumulate
    per group) so it consumes w2 groups at the rate they stream in.
  - single 8-buf PSUM pool shared by transposes/GEMM1/GEMM2; single xT
    buffer (the PE's program order makes chunk c+1's transposes wait for
    GEMM1 of chunk c, which is exactly when xT is dead anyway).

Compute dtype bf16 (fp32 PSUM accumulation), gelu on ScalarE matching
jax.nn.gelu(approximate=True): end-to-end rel-err ~3.4e-3.
"""

import numpy as np

import concourse.bacc as bacc
import concourse.mybir as mybir
import concourse.tile as tile
from concourse.bass_utils import run_bass_kernel_spmd
from concourse.masks import make_identity

E, B, N, D, F = 8, 2, 2048, 1024, 4096
TOK = B * N            # tokens per expert / per core
TC = 512               # token chunk processed per pipeline stage
NCHUNK = TOK // TC     # 8
P = 128
DO = D // P            # 8  d-tiles (GEMM1 contraction)
FO = F // P            # 32 f-tiles (GEMM2 contraction)
FG = 8                 # weight f-groups of 512 (4 f-tiles each)

F32 = mybir.dt.float32
BF16 = mybir.dt.bfloat16
GELU = mybir.ActivationFunctionType.Gelu_apprx_tanh


def _build_kernel(tc_ctx, nc, x, w1, b1, w2, b2, out):
    with (
        tc_ctx.tile_pool(name="wpool", bufs=1) as wp,
        tc_ctx.tile_pool(name="sscalar", bufs=4) as stg_a,
        tc_ctx.tile_pool(name="ssync", bufs=2) as stg_s,
        tc_ctx.tile_pool(name="xpool", bufs=1) as xp,
        tc_ctx.tile_pool(name="xbpool", bufs=2) as xbp,
        tc_ctx.tile_pool(name="xc1pool", bufs=2) as xc1p,
        tc_ctx.tile_pool(name="hpool", bufs=1) as hp,
        tc_ctx.tile_pool(name="opool", bufs=1) as op,
        tc_ctx.tile_pool(name="cpool", bufs=1) as cp,
        tc_ctx.tile_pool(name="dram", bufs=1, space="DRAM") as dp,
        tc_ctx.tile_pool(name="ps", bufs=8, space="PSUM") as psp,
    ):
        # identity for PE-mode transposes
        ident = cp.tile([P, P], F32, tag="ident")
        make_identity(nc, ident)
        identb = cp.tile([P, P], BF16, tag="identb")
        nc.vector.tensor_copy(identb, ident)

        # w1 tile (ki, do, fj) = w1[do*128+ki, fg*512+fj] : lhsT for GEMM1
        w1r = w1.rearrange("(do ki) f -> ki do f", ki=P)
        # w2 tile (ki, m, dj) = w2[fg*512 + m*128 + ki, dj] : rhs for GEMM2
        w2r = w2.rearrange("(fg m ki) d -> ki fg m d", ki=P, m=4)
        w1g = [
            wp.tile([P, DO, 512], BF16, tag=f"w1g{fg}", name=f"w1g{fg}")
            for fg in range(FG)
        ]
        w2g = [
            wp.tile([P, 4, D], BF16, tag=f"w2g{fg}", name=f"w2g{fg}")
            for fg in range(FG)
        ]

        # ---- weight loads: raw fp32 -> staging -> vector cast to bf16 ----
        # scalar HWDGE queue: w1 g0..g5 + w2 g3..g7 as [128, 512] eighths.
        # sync HWDGE queue:   w1 g6,g7 + w2 g0..g2 as [128, 1024] quarters
        # (after the chunk-0 x fp32 loads below).
        w1e = {}

        def w1_e8_dma(eng, pool, g, e8):
            t = pool.tile([P, 512], F32, tag="s8", name=f"w1s{g}_{e8}")
            eng.dma_start(t, w1r[:, e8, g * 512:(g + 1) * 512])
            w1e[(g, e8)] = t

        # w1 raw fp32: eighths e0-3 of every group on the scalar queue,
        # e4-7 on the sync queue (after its share of the chunk-0 x load) —
        # the two queues stream a group in parallel.
        for g in range(FG):
            for e8 in range(4):
                w1_e8_dma(nc.scalar, stg_a, g, e8)

        # w2 g3-g7: fp32->bf16 cast-DMA on SWDGE into DRAM scratch.  No PE
        # instruction may ever depend on the (slow, serial) SWDGE queue: the
        # tile scheduler hoists a coarse SWDGE wait to the head of the first
        # PE block, which was measured to stall the PE start to ~91us.  The
        # resident bf16 tiles are filled by fast HWDGE reloads below.
        w2d = {}
        for g in range(3, FG):
            t = dp.tile([P, 4, D], BF16, tag=f"w2d{g}", name=f"w2d{g}")
            nc.gpsimd.dma_start(t, w2r[:, g])
            w2d[g] = t

        # x chunks 2-7: fp32->bf16 cast-DMA into DRAM scratch on SWDGE,
        # behind w2 (not needed until ~186us); reloaded into SBUF halves by
        # HWDGE in the chunk loop.
        xd = [[None, None] for _ in range(NCHUNK)]
        for c in range(2, NCHUNK):
            for h in range(2):
                t = dp.tile([P, 2, D], BF16, tag=f"xd{c}_{h}", name=f"xd{c}_{h}")
                nc.gpsimd.dma_start(
                    t,
                    x[c * TC + h * 256:c * TC + (h + 1) * 256, :].rearrange(
                        "(tm p) d -> p tm d", p=P
                    ),
                )
                xd[c][h] = t

        # sync HWDGE queue: chunk-0 x fp32 quarters (128 tokens each), b1,
        # then the sync-side weight quarters, then b2.
        xq = {}
        for tm in range(4):
            for dh in range(2):
                t = stg_s.tile([P, 512], F32, tag="s8", name=f"xq{tm}_{dh}")
                nc.sync.dma_start(
                    t, x[tm * P:(tm + 1) * P, dh * 512:(dh + 1) * 512]
                )
                xq[(tm, dh)] = t

        # b1 on partitions (f-inner), one column per f-tile -> activation bias
        b1sb = cp.tile([P, FO], F32, tag="b1")
        nc.sync.dma_start(b1sb, b1.rearrange("(fo fi) -> fi fo", fi=P))

        for g in range(FG):
            for e8 in range(4, 8):
                w1_e8_dma(nc.sync, stg_s, g, e8)

        # b2 replicated across all 128 partitions (free dim = d); the
        # doubling chain sits behind w1 on the sync queue (needed ~124us).
        b2sb = cp.tile([P, D], F32, tag="b2")
        nc.sync.dma_start(b2sb[0:1, :], b2[None, :])
        k = 1
        while k < P:
            nc.sync.dma_start(b2sb[k:2 * k, :], b2sb[0:k, :])
            k *= 2

        # w2 g0-g2: raw fp32 eighths (e0-3 scalar / e4-7 sync), vector-cast
        # like w1 — ready by ~75us for the first GEMM2 groups.
        w2e = {}

        def w2_e8_dma(eng, pool, g, m, h2):
            t = pool.tile([P, 512], F32, tag="s8", name=f"w2s{g}_{m}_{h2}")
            eng.dma_start(t, w2r[:, g, m, h2 * 512:(h2 + 1) * 512])
            w2e[(g, m, h2)] = t

        for g in range(3):
            for m in range(4):
                for h2 in range(2):
                    w2_e8_dma(nc.scalar, stg_a, g, m, h2)

        # x chunk 1: raw fp32 eighths through a small dedicated pool, one
        # (tm, dh0)+(tm, dh1) pair in flight; PE-transposed in short
        # inserts between GEMM2-of-chunk-0 groups.
        xc1 = {}
        for tm in range(4):
            for dh in range(2):
                t = xc1p.tile([P, 512], F32, tag="c1", name=f"xc1_{tm}_{dh}")
                nc.sync.dma_start(
                    t,
                    x[TC + tm * P:TC + (tm + 1) * P, dh * 512:(dh + 1) * 512],
                )
                xc1[(tm, dh)] = t

        # w2 g3-g7: HWDGE reload of the SWDGE-cast DRAM scratch into the
        # resident bf16 tiles (each waits its background cast, done ~20-78us).
        for g in range(3, FG):
            nc.scalar.dma_start(w2g[g], w2d[g])

        # ---- weight casts on vector: w1 g0 first (GEMM1 needs it ~10us
        # in; the chunk-0 copies below land on vector right after) ----
        def cast_w1_e8(g):
            for e8 in range(8):
                nc.vector.tensor_copy(w1g[g][:, e8, :], w1e[(g, e8)])

        cast_w1_e8(0)

        # ---- chunk 0 transpose on PE (fp32 in, bf16 out via vector) ----
        xT = xp.tile([P, DO, TC], BF16, tag="xT")
        for tm in range(4):
            for dg in range(2):
                pt = psp.tile([P, 4, P], F32, tag="ps", name=f"psT{tm}_{dg}")
                for dj in range(4):
                    nc.tensor.transpose(
                        pt[:, dj, :],
                        xq[(tm, dg)][:, dj * P:(dj + 1) * P],
                        ident,
                    )
                nc.vector.tensor_copy(
                    xT[:, dg * 4:(dg + 1) * 4, tm * P:(tm + 1) * P], pt
                )

        # ---- remaining w1 + w2 g0-2 casts on vector, in arrival order ----
        for g in range(1, FG):
            cast_w1_e8(g)
        for g in range(3):
            for m in range(4):
                for h2 in range(2):
                    nc.vector.tensor_copy(
                        w2g[g][:, m, h2 * 512:(h2 + 1) * 512],
                        w2e[(g, m, h2)],
                    )

        # ---- main pipeline over token chunks ----
        xh = [[None, None] for _ in range(NCHUNK)]
        for c in range(NCHUNK):
            # HWDGE reload of chunk c+2's x halves from DRAM scratch (the
            # background SWDGE cast finished long before).
            if c + 2 < NCHUNK:
                for h in range(2):
                    t = xbp.tile([P, 2, D], BF16, tag="xh", name=f"xh{c + 2}_{h}")
                    eng = nc.scalar if h == 0 else nc.sync
                    eng.dma_start(t, xd[c + 2][h])
                    xh[c + 2][h] = t

            # GEMM1 + bias + gelu -> hT[f-part, fo, tok] (bf16)
            hT = hp.tile([P, FO, TC], BF16, tag="hT")
            for fo in range(FO):
                ps = psp.tile([P, TC], F32, tag="ps")
                w1t = w1g[fo // 4]
                fi = (fo % 4) * P
                for do in range(DO):
                    nc.tensor.matmul(
                        ps,
                        w1t[:, do, fi:fi + P],
                        xT[:, do, :],
                        start=(do == 0),
                        stop=(do == DO - 1),
                    )
                nc.scalar.activation(
                    hT[:, fo, :], ps, GELU, bias=b1sb[:, fo:fo + 1]
                )

            # chunk c+1 transposes on PE (bf16): xT is dead now (GEMM1 of
            # chunk c was its last reader, and the PE runs in order).
            # (chunk 1's transposes are fp32 inserts inside GEMM2-of-c0.)
            if 1 <= c < NCHUNK - 1:
                for h in range(2):
                    src = xh[c + 1][h]
                    for dg in range(2):
                        pt = psp.tile(
                            [P, 4, 256], BF16, tag="ps", name=f"ptb{h}_{dg}"
                        )
                        for dj in range(4):
                            do = dg * 4 + dj
                            for tm in range(2):
                                nc.tensor.transpose(
                                    pt[:, dj, tm * P:(tm + 1) * P],
                                    src[:, tm, do * P:(do + 1) * P],
                                    identb,
                                )
                        nc.scalar.activation(
                            xT[:, dg * 4:(dg + 1) * 4, h * 256:(h + 1) * 256],
                            pt,
                            mybir.ActivationFunctionType.Copy,
                        )

            # GEMM2 + bias -> out[tok, d] natural layout
            if c == 0:
                # f-group-major: all 8 psum tiles accumulate per group, so
                # matmuls consume w2 groups as they stream in.
                pts = [
                    psp.tile([P, 512], F32, tag="ps", name=f"ps2_{i}")
                    for i in range(8)
                ]
                for g in range(FG):
                    for tt in range(TC // P):
                        for dh in range(2):
                            pt = pts[tt * 2 + dh]
                            for j in range(4):
                                fo = g * 4 + j
                                nc.tensor.matmul(
                                    pt,
                                    hT[:, fo, tt * P:(tt + 1) * P],
                                    w2g[g][:, j, dh * 512:(dh + 1) * 512],
                                    start=(fo == 0),
                                    stop=(fo == FO - 1),
                                )
                    if 3 <= g <= 6:
                        # chunk-1 transpose insert (fp32): one tm block
                        tm = g - 3
                        for dg in range(2):
                            ptt = psp.tile(
                                [P, 4, P], F32, tag="ps", name=f"pc1_{tm}_{dg}"
                            )
                            for dj in range(4):
                                nc.tensor.transpose(
                                    ptt[:, dj, :],
                                    xc1[(tm, dg)][:, dj * P:(dj + 1) * P],
                                    ident,
                                )
                            nc.scalar.activation(
                                xT[:, dg * 4:(dg + 1) * 4, tm * P:(tm + 1) * P],
                                ptt,
                                mybir.ActivationFunctionType.Copy,
                            )
                for tt in range(TC // P):
                    for dh in range(2):
                        pt = pts[tt * 2 + dh]
                        osb = op.tile([P, 512], F32, tag="osb")
                        nc.vector.tensor_tensor(
                            osb, pt, b2sb[:, dh * 512:(dh + 1) * 512],
                            mybir.AluOpType.add,
                        )
                        row0 = c * TC + tt * P
                        nc.sync.dma_start(
                            out[row0:row0 + P, dh * 512:(dh + 1) * 512], osb
                        )
            else:
                for tt in range(TC // P):
                    for dh in range(2):
                        ps2t = psp.tile([P, 512], F32, tag="ps")
                        for fo in range(FO):
                            nc.tensor.matmul(
                                ps2t,
                                hT[:, fo, tt * P:(tt + 1) * P],
                                w2g[fo // 4][:, fo % 4, dh * 512:(dh + 1) * 512],
                                start=(fo == 0),
                                stop=(fo == FO - 1),
                            )
                        osb = op.tile([P, 512], F32, tag="osb")
                        nc.vector.tensor_tensor(
                            osb, ps2t, b2sb[:, dh * 512:(dh + 1) * 512],
                            mybir.AluOpType.add,
                        )
                        row0 = c * TC + tt * P
                        nc.sync.dma_start(
                            out[row0:row0 + P, dh * 512:(dh + 1) * 512], osb
                        )


_NC_CACHE = None


def _get_nc():
    global _NC_CACHE
    if _NC_CACHE is None:
        nc = bacc.Bacc(
            "TRN2", target_bir_lowering=False, num_devices=E, num_swdge_queues=4
        )
        x = nc.dram_tensor("x", [TOK, D], F32, kind="ExternalInput").ap()
        w1 = nc.dram_tensor("w1", [D, F], F32, kind="ExternalInput").ap()
        b1 = nc.dram_tensor("b1", [F], F32, kind="ExternalInput").ap()
        w2 = nc.dram_tensor("w2", [F, D], F32, kind="ExternalInput").ap()
        b2 = nc.dram_tensor("b2", [D], F32, kind="ExternalInput").ap()
        out = nc.dram_tensor("out", [TOK, D], F32, kind="ExternalOutput").ap()
        with tile.TileContext(nc) as tctx:
            _build_kernel(tctx, nc, x, w1, b1, w2, b2, out)
        nc.compile()
        _NC_CACHE = nc
    return _NC_CACHE


def kernel(run_opts=None, **inputs):
    x = np.ascontiguousarray(inputs["x"], dtype=np.float32)
    w1 = np.ascontiguousarray(inputs["w1"], dtype=np.float32)
    b1 = np.ascontiguousarray(inputs["b1"], dtype=np.float32)
    w2 = np.ascontiguousarray(inputs["w2"], dtype=np.float32)
    b2 = np.ascontiguousarray(inputs["b2"], dtype=np.float32)

    # x.view(E, B, N, D) in the reference is a pure reshape: expert e owns the
    # contiguous token block e of the flattened (E*B*N, D) buffer.
    xf = x.reshape(E, TOK, D)
    in_maps = [
        {"x": xf[e], "w1": w1[e], "b1": b1[e], "w2": w2[e], "b2": b2[e]}
        for e in range(E)
    ]
    nc = _get_nc()
    res = run_bass_kernel_spmd(
        nc, in_maps, core_ids=list(range(E)), **(run_opts or {})
    )
    outs = np.stack([res.results[e]["out"] for e in range(E)])  # (E, TOK, D)
    if run_opts:
        kernel.last_results = res
    # outputs.view(B, E, N, D) in the reference: reinterpret (E, B*N, D) buffer
    return outs.reshape(B, E, N, D)


# revision 23
# speedup vs baseline: 1.0357x; 1.0357x over previous
"""MoE grouped-GEMM expert MLP for Trainium2, expert-parallel over 8 NeuronCores.

Problem: x:(B=2, E=8, N=2048, D=1024), per-expert 2-layer GELU MLP with
w1:(E, D, F=4096), w2:(E, F, D).  Reference computes
  xe = x.reshape(E, B*N, D)          # pure buffer reinterpretation
  h  = gelu_tanh(xe @ w1 + b1)
  out= h @ w2 + b2                   # reshaped back to (B, E, N, D)

Sharding: expert parallelism -- core e runs expert e on its contiguous
token block xe[e] (4096 tokens).  No collectives needed.

Per-core layout: hidden activations kept transposed ("hT" = [f, tok]) so both
weight matrices are consumed in their NATIVE layouts:
  GEMM1: hT[f,tok]  = (w1[d,f] as lhsT).T @ xT[d,tok]
  GEMM2: out[tok,d] = (hT[f,tok] slice as lhsT).T @ w2[f,d]

Hard-won profiling facts this schedule is built around:
  - DMA packets that cast fp32->bf16 in flight run at roughly HALF the DMA
    engine byte rate; XBAR DMA-transposes are far slower still (~10-13 GB/s),
    so every chunk's xT is produced by PE-mode transposes (~1.7-3.4us/chunk
    of PE time) instead of DMA transposes.
  - The two HWDGE queues (sync + scalar) each sustain ~165 GB/s when both
    are busy; the serial SWDGE queue ~150 GB/s of cast work.  The first
    ~125us is DMA-bound: w1 (needed by ~65us) streams as raw fp32 quarters
    split across both HWDGE queues and is cast to bf16 by the Vector
    engine; w2 g0-2 (needed ~77-91us) likewise; w2 g3-7 (needed ~97-124us)
    is cast fp32->bf16 in the background on SWDGE into DRAM scratch and
    re-loaded raw by HWDGE just in time.  x chunks 2-7 take the same
    background-cast path (needed from ~186us on).
  - The tile scheduler maps DMA completions onto a small set of counting
    semaphores in EMISSION order, so a consumer's wait can only be
    fine-grained if its producer is emitted no later than unrelated
    slower traffic.  All DMAs below are emitted in need order, and no PE
    instruction ever depends on the slow SWDGE stream.
  - GEMM2 of chunk 0 is emitted f-group-major (all 8 PSUM tiles accumulate
    per group) so it consumes w2 groups at the rate they stream in, with
    chunk 1's PE transposes (raw fp32 staged on HWDGE) inserted between
    groups 3-6.

Compute dtype bf16 (fp32 PSUM accumulation), gelu on ScalarE matching
jax.nn.gelu(approximate=True): end-to-end rel-err ~3.4e-3.
"""

import numpy as np

import concourse.bacc as bacc
import concourse.mybir as mybir
import concourse.tile as tile
from concourse.bass_utils import run_bass_kernel_spmd
from concourse.masks import make_identity

E, B, N, D, F = 8, 2, 2048, 1024, 4096
TOK = B * N            # tokens per expert / per core
TC = 512               # token chunk processed per pipeline stage
NCHUNK = TOK // TC     # 8
P = 128
DO = D // P            # 8  d-tiles (GEMM1 contraction)
FO = F // P            # 32 f-tiles (GEMM2 contraction)
FG = 8                 # weight f-groups of 512 (4 f-tiles each)

F32 = mybir.dt.float32
BF16 = mybir.dt.bfloat16
GELU = mybir.ActivationFunctionType.Gelu_apprx_tanh
COPY = mybir.ActivationFunctionType.Copy


def _build_kernel(tc_ctx, nc, x, w1, b1, w2, b2, out):
    with (
        tc_ctx.tile_pool(name="wpool", bufs=1) as wp,
        tc_ctx.tile_pool(name="sscalar", bufs=2) as stg_a,
        tc_ctx.tile_pool(name="ssync", bufs=2) as stg_s,
        tc_ctx.tile_pool(name="xpool", bufs=1) as xp,
        tc_ctx.tile_pool(name="xbpool", bufs=2) as xbp,
        tc_ctx.tile_pool(name="hpool", bufs=1) as hp,
        tc_ctx.tile_pool(name="opool", bufs=2) as op,
        tc_ctx.tile_pool(name="cpool", bufs=1) as cp,
        tc_ctx.tile_pool(name="dram", bufs=1, space="DRAM") as dp,
        tc_ctx.tile_pool(name="ps", bufs=8, space="PSUM") as psp,
    ):
        # identities for PE-mode transposes (fp32 for staged x, bf16 for
        # background-cast x)
        ident = cp.tile([P, P], F32, tag="ident")
        make_identity(nc, ident)
        identb = cp.tile([P, P], BF16, tag="identb")
        nc.vector.tensor_copy(identb, ident)

        # w1 tile (ki, do, fj) = w1[do*128+ki, fg*512+fj] : lhsT for GEMM1
        w1r = w1.rearrange("(do ki) f -> ki do f", ki=P)
        # w2 tile (ki, m, dj) = w2[fg*512 + m*128 + ki, dj] : rhs for GEMM2
        w2r = w2.rearrange("(fg m ki) d -> ki fg m d", ki=P, m=4)
        w1g = [
            wp.tile([P, DO, 512], BF16, tag=f"w1g{fg}", name=f"w1g{fg}")
            for fg in range(FG)
        ]
        w2g = [
            wp.tile([P, 4, D], BF16, tag=f"w2g{fg}", name=f"w2g{fg}")
            for fg in range(FG)
        ]

        # ================= DMA emission, strictly in need order ============
        # -- x chunk 0: four fp32 quarters, sync queue (needed ~8us) --
        xq = []
        for tm in range(4):
            t = stg_s.tile([P, D], F32, tag="sq", name=f"xq{tm}")
            nc.sync.dma_start(t, x[tm * P:(tm + 1) * P, :])
            xq.append(t)

        # -- b1 (needed ~13us) --
        b1sb = cp.tile([P, FO], F32, tag="b1")
        nc.sync.dma_start(b1sb, b1.rearrange("(fo fi) -> fi fo", fi=P))

        # -- w1 raw fp32 quarters, 2 per group on each queue (needed 17-65us),
        #    vector-cast to bf16 group by group --
        def w1_q_dma(eng, pool, g, q):
            t = pool.tile([P, D], F32, tag="sq", name=f"w1s{g}_{q}")
            eng.dma_start(
                t.rearrange("p (do f) -> p do f", do=2),
                w1r[:, 2 * q:2 * q + 2, g * 512:(g + 1) * 512],
            )
            return t

        def cast_w1_q(g, q, t):
            nc.vector.tensor_copy(
                w1g[g][:, 2 * q:2 * q + 2, :],
                t.rearrange("p (do f) -> p do f", do=2),
            )

        for g in range(FG):
            ts = [
                w1_q_dma(nc.scalar, stg_a, g, 0),
                w1_q_dma(nc.sync, stg_s, g, 2),
                w1_q_dma(nc.scalar, stg_a, g, 1),
                w1_q_dma(nc.sync, stg_s, g, 3),
            ]
            for q, t in zip((0, 2, 1, 3), ts):
                cast_w1_q(g, q, t)
            if g == 0:
                # chunk-0 PE transposes + their vector copies land here in
                # the vector stream: right after w1 g0's casts.
                xT = xp.tile([P, DO, TC], BF16, tag="xT")
                for tm in range(4):
                    for dg in range(2):
                        pt = psp.tile(
                            [P, 4, P], F32, tag="ps", name=f"psT{tm}_{dg}"
                        )
                        for dj in range(4):
                            do = dg * 4 + dj
                            nc.tensor.transpose(
                                pt[:, dj, :],
                                xq[tm][:, do * P:(do + 1) * P],
                                ident,
                            )
                        nc.vector.tensor_copy(
                            xT[:, dg * 4:(dg + 1) * 4, tm * P:(tm + 1) * P],
                            pt,
                        )

        # -- b2 row + replication chain, sync (needed ~124us, cheap) --
        b2sb = cp.tile([P, D], F32, tag="b2")
        nc.sync.dma_start(b2sb[0:1, :], b2[None, :])
        k = 1
        while k < P:
            nc.sync.dma_start(b2sb[k:2 * k, :], b2sb[0:k, :])
            k *= 2

        # -- w2 g0-2 raw fp32 quarters (needed 77-91us) + casts --
        w2s = {}
        for g in range(3):
            for m in range(4):
                eng, pool = (
                    (nc.scalar, stg_a) if m % 2 == 0 else (nc.sync, stg_s)
                )
                t = pool.tile([P, D], F32, tag="sq", name=f"w2s{g}_{m}")
                eng.dma_start(t, w2r[:, g, m, :])
                w2s[(g, m)] = t
            for m in range(4):
                nc.vector.tensor_copy(w2g[g][:, m, :], w2s[(g, m)])

        # -- x chunk 1 raw fp32 quarters (needed ~104-118us); these are the
        #    LAST tenants of the staging pools, so holding their slots until
        #    the mid-GEMM2 transpose inserts blocks nothing --
        xc1 = []
        for tm in range(4):
            eng, pool = (
                (nc.scalar, stg_a) if tm % 2 == 0 else (nc.sync, stg_s)
            )
            t = pool.tile([P, D], F32, tag="sq", name=f"xc1_{tm}")
            eng.dma_start(t, x[TC + tm * P:TC + (tm + 1) * P, :])
            xc1.append(t)

        # -- background SWDGE casts into DRAM scratch: w2 g3-7 (reloaded
        #    just-in-time below), then x chunks 2-7.  No PE instruction
        #    depends on these DMAs. --
        w2d = {}
        for g in range(3, FG):
            t = dp.tile([P, 4, D], BF16, tag=f"w2d{g}", name=f"w2d{g}")
            nc.gpsimd.dma_start(t, w2r[:, g])
            w2d[g] = t
        xd = [[None, None] for _ in range(NCHUNK)]
        for c in range(2, NCHUNK):
            for h in range(2):
                t = dp.tile(
                    [P, 2, D], BF16, tag=f"xd{c}_{h}", name=f"xd{c}_{h}"
                )
                nc.gpsimd.dma_start(
                    t,
                    x[c * TC + h * 256:c * TC + (h + 1) * 256, :].rearrange(
                        "(tm p) d -> p tm d", p=P
                    ),
                )
                xd[c][h] = t

        # -- w2 g3-7: HWDGE raw reload of the background casts (scalar) --
        for g in range(3, FG):
            nc.scalar.dma_start(w2g[g], w2d[g])

        # ================= main pipeline over token chunks =================
        xh = [[None, None] for _ in range(NCHUNK)]
        for c in range(NCHUNK):
            # HWDGE reload of chunk c+2's x halves from DRAM scratch
            if c + 2 < NCHUNK:
                for h in range(2):
                    t = xbp.tile(
                        [P, 2, D], BF16, tag="xh", name=f"xh{c + 2}_{h}"
                    )
                    eng = nc.scalar if h == 0 else nc.sync
                    eng.dma_start(t, xd[c + 2][h])
                    xh[c + 2][h] = t

            # GEMM1 + bias + gelu -> hT[f-part, fo, tok] (bf16)
            hT = hp.tile([P, FO, TC], BF16, tag="hT")
            for fo in range(FO):
                ps = psp.tile([P, TC], F32, tag="ps")
                w1t = w1g[fo // 4]
                fi = (fo % 4) * P
                for do in range(DO):
                    nc.tensor.matmul(
                        ps,
                        w1t[:, do, fi:fi + P],
                        xT[:, do, :],
                        start=(do == 0),
                        stop=(do == DO - 1),
                    )
                nc.scalar.activation(
                    hT[:, fo, :], ps, GELU, bias=b1sb[:, fo:fo + 1]
                )

            # chunk c+1 transposes on PE (bf16 path, chunks 2-7): xT is
            # dead now (GEMM1 of chunk c was its last reader, PE in order).
            if 1 <= c < NCHUNK - 1:
                for h in range(2):
                    src = xh[c + 1][h]
                    for dg in range(2):
                        pt = psp.tile(
                            [P, 4, 256], BF16, tag="ps", name=f"ptb{h}_{dg}"
                        )
                        for dj in range(4):
                            do = dg * 4 + dj
                            for tm in range(2):
                                nc.tensor.transpose(
                                    pt[:, dj, tm * P:(tm + 1) * P],
                                    src[:, tm, do * P:(do + 1) * P],
                                    identb,
                                )
                        nc.scalar.activation(
                            xT[:, dg * 4:(dg + 1) * 4, h * 256:(h + 1) * 256],
                            pt,
                            COPY,
                        )

            # GEMM2 + bias -> out[tok, d] natural layout
            if c == 0:
                # f-group-major: all 8 psum tiles accumulate per group so
                # matmuls consume w2 groups at the rate they stream in;
                # chunk-1 transposes (fp32) inserted after groups 3-6.
                pts = [
                    psp.tile([P, 512], F32, tag="ps", name=f"ps2_{i}")
                    for i in range(8)
                ]
                for g in range(FG):
                    for tt in range(TC // P):
                        for dh in range(2):
                            pt = pts[tt * 2 + dh]
                            for j in range(4):
                                fo = g * 4 + j
                                nc.tensor.matmul(
                                    pt,
                                    hT[:, fo, tt * P:(tt + 1) * P],
                                    w2g[g][:, j, dh * 512:(dh + 1) * 512],
                                    start=(fo == 0),
                                    stop=(fo == FO - 1),
                                )
                    if 3 <= g <= 6:
                        tm = g - 3
                        for dg in range(2):
                            ptt = psp.tile(
                                [P, 4, P], F32, tag="ps", name=f"pc1_{tm}_{dg}"
                            )
                            for dj in range(4):
                                do = dg * 4 + dj
                                nc.tensor.transpose(
                                    ptt[:, dj, :],
                                    xc1[tm][:, do * P:(do + 1) * P],
                                    ident,
                                )
                            nc.scalar.activation(
                                xT[:, dg * 4:(dg + 1) * 4, tm * P:(tm + 1) * P],
                                ptt,
                                COPY,
                            )
                for tt in range(TC // P):
                    for dh in range(2):
                        pt = pts[tt * 2 + dh]
                        osb = op.tile([P, 512], F32, tag="osb")
                        nc.vector.tensor_tensor(
                            osb, pt, b2sb[:, dh * 512:(dh + 1) * 512],
                            mybir.AluOpType.add,
                        )
                        row0 = c * TC + tt * P
                        nc.sync.dma_start(
                            out[row0:row0 + P, dh * 512:(dh + 1) * 512], osb
                        )
            else:
                for tt in range(TC // P):
                    for dh in range(2):
                        ps2t = psp.tile([P, 512], F32, tag="ps")
                        for fo in range(FO):
                            nc.tensor.matmul(
                                ps2t,
                                hT[:, fo, tt * P:(tt + 1) * P],
                                w2g[fo // 4][:, fo % 4, dh * 512:(dh + 1) * 512],
                                start=(fo == 0),
                                stop=(fo == FO - 1),
                            )
                        osb = op.tile([P, 512], F32, tag="osb")
                        nc.vector.tensor_tensor(
                            osb, ps2t, b2sb[:, dh * 512:(dh + 1) * 512],
                            mybir.AluOpType.add,
                        )
                        row0 = c * TC + tt * P
                        nc.sync.dma_start(
                            out[row0:row0 + P, dh * 512:(dh + 1) * 512], osb
                        )


_NC_CACHE = None


def _get_nc():
    global _NC_CACHE
    if _NC_CACHE is None:
        nc = bacc.Bacc(
            "TRN2", target_bir_lowering=False, num_devices=E, num_swdge_queues=4
        )
        x = nc.dram_tensor("x", [TOK, D], F32, kind="ExternalInput").ap()
        w1 = nc.dram_tensor("w1", [D, F], F32, kind="ExternalInput").ap()
        b1 = nc.dram_tensor("b1", [F], F32, kind="ExternalInput").ap()
        w2 = nc.dram_tensor("w2", [F, D], F32, kind="ExternalInput").ap()
        b2 = nc.dram_tensor("b2", [D], F32, kind="ExternalInput").ap()
        out = nc.dram_tensor("out", [TOK, D], F32, kind="ExternalOutput").ap()
        with tile.TileContext(nc) as tctx:
            _build_kernel(tctx, nc, x, w1, b1, w2, b2, out)
        nc.compile()
        _NC_CACHE = nc
    return _NC_CACHE


def kernel(run_opts=None, **inputs):
    x = np.ascontiguousarray(inputs["x"], dtype=np.float32)
    w1 = np.ascontiguousarray(inputs["w1"], dtype=np.float32)
    b1 = np.ascontiguousarray(inputs["b1"], dtype=np.float32)
    w2 = np.ascontiguousarray(inputs["w2"], dtype=np.float32)
    b2 = np.ascontiguousarray(inputs["b2"], dtype=np.float32)

    # x.view(E, B, N, D) in the reference is a pure reshape: expert e owns the
    # contiguous token block e of the flattened (E*B*N, D) buffer.
    xf = x.reshape(E, TOK, D)
    in_maps = [
        {"x": xf[e], "w1": w1[e], "b1": b1[e], "w2": w2[e], "b2": b2[e]}
        for e in range(E)
    ]
    nc = _get_nc()
    res = run_bass_kernel_spmd(
        nc, in_maps, core_ids=list(range(E)), **(run_opts or {})
    )
    outs = np.stack([res.results[e]["out"] for e in range(E)])  # (E, TOK, D)
    if run_opts:
        kernel.last_results = res
    # outputs.view(B, E, N, D) in the reference: reinterpret (E, B*N, D) buffer
    return outs.reshape(B, E, N, D)
